# revision 14
# baseline (speedup 1.0000x reference)
"""MHA kernel for 8 Trainium2 NeuronCores.

Reference computation (per batch b):
    Qh = (q[b] @ Wq.T) * Dh^-0.5, Kh = k[b] @ Wk.T, Vh = v[b] @ Wv.T   (split into 16 heads of 128)
    P  = softmax(Qh Kh^T), O = P Vh, out[b] = concat_heads(O) @ Wo.T
Mask is all-False (spec fill=zeros) and is ignored.

Sharding: 8 cores = 2 batches x 4 head-groups (4 heads / core).
Wq/Wk/Wv are split column-wise (output dims), Wo row-wise (input dims);
the all-reduce after the output projection is done on the host during the
gather (sum of the 4 per-head-group partial projections per batch).

Per-core device kernel (all matmul operands bf16, PSUM accumulation fp32):
  inputs (host-prepared): xq/xk/xv = x[b].T [D,S]; wq/wk/wv = W_slice.T [D,512]
  (Dh^-0.5 folded into wq); wo = Wo_slice.T [512, D].
  1) QhT/KhT [Dh,S] per head (head-dim on partitions), Vh [S, 512] (seq on partitions)
  2) per head: scores^T [Sk,Sq] = KhT_m^T.T @ QhT ; P^T = exp(scores^T)
     O^T [Dh,Sq] = sum_m Vh_m.T @ P^T_m ; denom = sum_m ones.T @ P^T_m (M=1 matmuls)
     denom broadcast across partitions via k=1 matmul, reciprocal, normalize O^T
  3) partial out = concat(O^T).T @ wo, accumulated over the 4 heads in PSUM
"""

import numpy as np
import ml_dtypes

BF16 = ml_dtypes.bfloat16

B = 2
S = 2048
D = 2048
NH_TOT = 16
DH = 128
H = 4            # heads per core
HS = H * DH      # 512, model-dim slice per core
P = 128
KD = D // P      # 16 contraction tiles over model dim
MT = S // P      # 16 seq tiles
N4 = S // 512    # 4 column groups of 512

_CACHE: dict = {}


def _build_bass():
    import concourse.tile as tile
    from concourse import bacc, mybir

    f32 = mybir.dt.float32
    bf16 = mybir.dt.bfloat16
    Exp = mybir.ActivationFunctionType.Exp

    nc = bacc.Bacc()

    xq = nc.declare_dram_parameter("xq", [D, S], bf16, isOutput=False)
    xk = nc.declare_dram_parameter("xk", [D, S], bf16, isOutput=False)
    xv = nc.declare_dram_parameter("xv", [D, S], bf16, isOutput=False)
    wq = nc.declare_dram_parameter("wq", [D, HS], bf16, isOutput=False)
    wk = nc.declare_dram_parameter("wk", [D, HS], bf16, isOutput=False)
    wv = nc.declare_dram_parameter("wv", [D, HS], bf16, isOutput=False)
    wo = nc.declare_dram_parameter("wo", [HS, D], bf16, isOutput=False)
    out = nc.declare_dram_parameter("out", [S, D], f32, isOutput=True)

    dma = nc.default_dma_engine

    with tile.TileContext(nc) as tc:
        with (
            tc.sbuf_pool(name="const", bufs=1) as cpool,
            tc.sbuf_pool(name="persist", bufs=1) as ppool,
            tc.sbuf_pool(name="small", bufs=4) as spool,
            tc.sbuf_pool(name="ostage", bufs=4) as opool,
        ):
            ones = cpool.tile([P, P], bf16, tag="ones")
            nc.vector.memset(ones, 1.0)

            qhT = ppool.tile([P, H, S], bf16, tag="qhT")   # [Dh, h, Sq]
            khT = ppool.tile([P, H, S], bf16, tag="khT")   # [Dh, h, Sk]
            vh = ppool.tile([P, MT, HS], bf16, tag="vh")   # [seq_p, m, 4*Dh]
            oT = ppool.tile([P, H, S], bf16, tag="oT")     # [Dh, h, Sq] normalized
            wo_sb = ppool.tile([P, H, D], bf16, tag="wo_sb")
            dma.dma_start(wo_sb, wo.rearrange("(k p) n -> p k n", p=P))

            # ---------------- projections ----------------
            with (
                tc.sbuf_pool(name="wqkv", bufs=1) as wpool,
                tc.sbuf_pool(name="xs", bufs=20) as xpool,
                tc.psum_pool(name="pproj", bufs=8) as pjp,
            ):
                wq_sb = wpool.tile([P, KD, HS], bf16, tag="wq_sb")
                wk_sb = wpool.tile([P, KD, HS], bf16, tag="wk_sb")
                wv_sb = wpool.tile([P, KD, HS], bf16, tag="wv_sb")
                dma.dma_start(wq_sb, wq.rearrange("(k p) n -> p k n", p=P))
                dma.dma_start(wk_sb, wk.rearrange("(k p) n -> p k n", p=P))
                dma.dma_start(wv_sb, wv.rearrange("(k p) n -> p k n", p=P))

                def proj_qk(x_dram, w_sb, out_sb):
                    # out_sb[:, h, :] = (x^T)^T-contraction: for each head dim tile
                    # lhsT = w_sb[:, kd, h*128:(h+1)*128], rhs = x^T k-slice
                    for nh in range(2):  # S halves, 1024 wide
                        xt = []
                        for kd in range(KD):
                            xti = xpool.tile([P, 1024], bf16, tag="xt")
                            dma.dma_start(
                                xti,
                                x_dram[kd * P:(kd + 1) * P, nh * 1024:(nh + 1) * 1024],
                            )
                            xt.append(xti)
                        pss = [
                            [pjp.tile([P, 512], f32, tag="psproj", name="psproj") for _ in range(2)]
                            for _ in range(H)
                        ]
                        for kd in range(KD):
                            for h in range(H):
                                for n in range(2):
                                    nc.tensor.matmul(
                                        pss[h][n],
                                        lhsT=w_sb[:, kd, h * P:(h + 1) * P],
                                        rhs=xt[kd][:, n * 512:(n + 1) * 512],
                                        start=(kd == 0),
                                        stop=(kd == KD - 1),
                                    )
                        for h in range(H):
                            for n in range(2):
                                dst = out_sb[:, h, nh * 1024 + n * 512: nh * 1024 + (n + 1) * 512]
                                if (h * 2 + n) % 2 == 0:
                                    nc.scalar.copy(dst, pss[h][n])
                                else:
                                    nc.vector.tensor_copy(dst, pss[h][n])

                proj_qk(xq, wq_sb, qhT)
                proj_qk(xk, wk_sb, khT)

                # V projection: Vh [seq, 512]; lhsT = xv^T tile (stationary)
                for nh in range(2):  # seq halves
                    xt2 = []
                    for kd in range(KD):
                        xti = xpool.tile([P, 1024], bf16, tag="xt")
                        dma.dma_start(
                            xti,
                            xv[kd * P:(kd + 1) * P, nh * 1024:(nh + 1) * 1024],
                        )
                        xt2.append(xti)
                    for mg in range(8):
                        m = nh * 8 + mg
                        psv = pjp.tile([P, 512], f32, tag="psproj")
                        for kd in range(KD):
                            nc.tensor.matmul(
                                psv,
                                lhsT=xt2[kd][:, mg * P:(mg + 1) * P],
                                rhs=wv_sb[:, kd, :],
                                start=(kd == 0),
                                stop=(kd == KD - 1),
                            )
                        if m % 2 == 0:
                            nc.scalar.copy(vh[:, m, :], psv)
                        else:
                            nc.vector.tensor_copy(vh[:, m, :], psv)

            # ---------------- attention (per head) ----------------
            with (
                tc.sbuf_pool(name="pts", bufs=24) as ptpool,
                tc.psum_pool(name="pattn", bufs=1) as pap,
            ):
                def normalize(hh, ps_o_hh, ps_d_hh):
                    # oT = ps_o * (1/denom) broadcast across partitions
                    for n in range(N4):
                        d_bf = spool.tile([1, 512], bf16, tag="d_bf")
                        nc.scalar.copy(d_bf, ps_d_hh[32 * n:32 * n + 1, :])
                        ps_b = pap.tile([P, 512], f32, tag="ps_b", bufs=1)
                        nc.tensor.matmul(ps_b, lhsT=ones[0:1, :], rhs=d_bf)
                        rb = spool.tile([P, 512], f32, tag="rb")
                        nc.vector.reciprocal(rb, ps_b)
                        nc.vector.tensor_mul(
                            oT[:, hh, n * 512:(n + 1) * 512], ps_o_hh[n], rb
                        )

                for h in range(H):
                    pt = []  # P^T tiles [Sk_tile, Sq]
                    ps_o = [pap.tile([P, 512], f32, tag=f"ps_o{n}", bufs=1, name=f"ps_o{n}") for n in range(N4)]
                    ps_d = pap.tile([P, 512], f32, tag="ps_d", bufs=1)

                    def scores_half(m, pti, nlo):
                        for n in (nlo, nlo + 1):
                            ps_s = pap.tile([P, 512], f32, tag="ps_s", bufs=2)
                            nc.tensor.matmul(
                                ps_s,
                                lhsT=khT[:, h, m * P:(m + 1) * P],
                                rhs=qhT[:, h, n * 512:(n + 1) * 512],
                            )
                            nc.scalar.activation(
                                pti[:, n * 512:(n + 1) * 512], ps_s, Exp
                            )

                    # software-pipelined: scores/exp for tile m, O/denom for m-2
                    # (interleaved so PE never waits on ACT draining score PSUM)
                    for mstep in range(MT + 2):
                        if mstep < MT:
                            pti = ptpool.tile([P, S], bf16, tag="pt")
                            scores_half(mstep, pti, 0)
                            pt.append(pti)
                        if mstep >= 2:
                            m = mstep - 2
                            for n in range(N4):
                                nc.tensor.matmul(
                                    ps_o[n],
                                    lhsT=vh[:, m, h * P:(h + 1) * P],
                                    rhs=pt[m][:, n * 512:(n + 1) * 512],
                                    start=(m == 0),
                                    stop=(m == MT - 1),
                                )
                        if mstep < MT:
                            scores_half(mstep, pt[mstep], 2)
                        if mstep >= 2:
                            m = mstep - 2
                            for n in range(N4):
                                nc.tensor.matmul(
                                    ps_d[32 * n:32 * n + 1, :],
                                    lhsT=ones[:, 0:1],
                                    rhs=pt[m][:, n * 512:(n + 1) * 512],
                                    start=(m == 0),
                                    stop=(m == MT - 1),
                                    tile_position=(0, 32 * n),
                                )
                    normalize(h, ps_o, ps_d)

            # ---------------- output projection ----------------
            with tc.psum_pool(name="pout", bufs=8) as pop:
                for m in range(MT):
                    psf = [pop.tile([P, 512], f32, tag="psout", name="psout") for _ in range(N4)]
                    for kh in range(H):
                        for n in range(N4):
                            nc.tensor.matmul(
                                psf[n],
                                lhsT=oT[:, kh, m * P:(m + 1) * P],
                                rhs=wo_sb[:, kh, n * 512:(n + 1) * 512],
                                start=(kh == 0),
                                stop=(kh == H - 1),
                            )
                    for n in range(N4):
                        ob = opool.tile([P, 512], f32, tag="ob")
                        if n % 2 == 0:
                            nc.scalar.copy(ob, psf[n])
                        else:
                            nc.vector.tensor_copy(ob, psf[n])
                        dma.dma_start(
                            out[m * P:(m + 1) * P, n * 512:(n + 1) * 512], ob
                        )

    nc.compile()
    return nc


def _get_nc():
    if "nc" not in _CACHE:
        _CACHE["nc"] = _build_bass()
    return _CACHE["nc"]


def _prep_inputs(q, k, v, Wq, Wk, Wv, Wo):
    """Host-side sharding: per-core transposed bf16 slices."""
    scale = float(DH) ** -0.5
    q = np.asarray(q, np.float32)
    k = np.asarray(k, np.float32)
    v = np.asarray(v, np.float32)
    Wq = np.asarray(Wq, np.float32)
    Wk = np.asarray(Wk, np.float32)
    Wv = np.asarray(Wv, np.float32)
    Wo = np.asarray(Wo, np.float32)
    in_maps = []
    xT = {}
    for b in range(B):
        xT[b] = (
            q[b].T.astype(BF16),
            k[b].T.astype(BF16),
            v[b].T.astype(BF16),
        )
    for c in range(8):
        b, hg = divmod(c, 4)
        hs = hg * HS
        xqT, xkT, xvT = xT[b]
        in_maps.append(
            {
                "xq": xqT,
                "xk": xkT,
                "xv": xvT,
                "wq": np.ascontiguousarray((Wq[hs:hs + HS, :] * scale).T).astype(BF16),
                "wk": np.ascontiguousarray(Wk[hs:hs + HS, :].T).astype(BF16),
                "wv": np.ascontiguousarray(Wv[hs:hs + HS, :].T).astype(BF16),
                "wo": np.ascontiguousarray(Wo[:, hs:hs + HS].T).astype(BF16),
            }
        )
    return in_maps


def run_spmd(q, k, v, Wq, Wk, Wv, Wo, trace=False):
    from concourse.bass_utils import run_bass_kernel_spmd

    nc = _get_nc()
    in_maps = _prep_inputs(q, k, v, Wq, Wk, Wv, Wo)
    res = run_bass_kernel_spmd(nc, in_maps, list(range(8)), trace=trace)
    out = np.zeros((B, S, D), np.float32)
    for c in range(8):
        out[c // 4] += np.asarray(res.results[c]["out"], np.float32)
    return out, res


def kernel(q, k, v, mask, Wq, Wk, Wv, Wo):
    out, _ = run_spmd(q, k, v, Wq, Wk, Wv, Wo, trace=False)
    return out


# revision 15
# speedup vs baseline: 1.0148x; 1.0148x over previous
"""MHA kernel for 8 Trainium2 NeuronCores.

Reference computation (per batch b):
    Qh = (q[b] @ Wq.T) * Dh^-0.5, Kh = k[b] @ Wk.T, Vh = v[b] @ Wv.T   (split into 16 heads of 128)
    P  = softmax(Qh Kh^T), O = P Vh, out[b] = concat_heads(O) @ Wo.T
Mask is all-False (spec fill=zeros) and is ignored.

Sharding: 8 cores = 2 batches x 4 head-groups (4 heads / core).
Wq/Wk/Wv are split column-wise (output dims), Wo row-wise (input dims);
the all-reduce after the output projection is done on the host during the
gather (sum of the 4 per-head-group partial projections per batch).

Per-core device kernel (all matmul operands bf16, PSUM accumulation fp32):
  inputs (host-prepared): xq/xk/xv = x[b].T [D,S]; wq/wk/wv = W_slice.T [D,512]
  (Dh^-0.5 folded into wq); wo = Wo_slice.T [512, D].
  1) QhT/KhT [Dh,S] per head (head-dim on partitions), Vh [S, 512] (seq on partitions)
  2) per head: scores^T [Sk,Sq] = KhT_m^T.T @ QhT ; P^T = exp(scores^T)
     O^T [Dh,Sq] = sum_m Vh_m.T @ P^T_m ; denom = sum_m ones.T @ P^T_m (M=1 matmuls)
     denom broadcast across partitions via k=1 matmul, reciprocal, normalize O^T
  3) partial out = concat(O^T).T @ wo, accumulated over the 4 heads in PSUM
"""

import numpy as np
import ml_dtypes

BF16 = ml_dtypes.bfloat16

B = 2
S = 2048
D = 2048
NH_TOT = 16
DH = 128
H = 4            # heads per core
HS = H * DH      # 512, model-dim slice per core
P = 128
KD = D // P      # 16 contraction tiles over model dim
MT = S // P      # 16 seq tiles
N4 = S // 512    # 4 column groups of 512

_CACHE: dict = {}


def _build_bass():
    import concourse.tile as tile
    from concourse import bacc, mybir

    f32 = mybir.dt.float32
    bf16 = mybir.dt.bfloat16
    Exp = mybir.ActivationFunctionType.Exp

    nc = bacc.Bacc()

    xq = nc.declare_dram_parameter("xq", [D, S], bf16, isOutput=False)
    xk = nc.declare_dram_parameter("xk", [D, S], bf16, isOutput=False)
    xv = nc.declare_dram_parameter("xv", [D, S], bf16, isOutput=False)
    wq = nc.declare_dram_parameter("wq", [D, HS], bf16, isOutput=False)
    wk = nc.declare_dram_parameter("wk", [D, HS], bf16, isOutput=False)
    wv = nc.declare_dram_parameter("wv", [D, HS], bf16, isOutput=False)
    wo = nc.declare_dram_parameter("wo", [HS, D], bf16, isOutput=False)
    out = nc.declare_dram_parameter("out", [S, D], f32, isOutput=True)

    dma = nc.default_dma_engine

    with tile.TileContext(nc) as tc:
        with (
            tc.sbuf_pool(name="const", bufs=1) as cpool,
            tc.sbuf_pool(name="persist", bufs=1) as ppool,
            tc.sbuf_pool(name="small", bufs=4) as spool,
            tc.sbuf_pool(name="ostage", bufs=4) as opool,
        ):
            ones = cpool.tile([P, P], bf16, tag="ones")
            nc.vector.memset(ones, 1.0)

            qhT = ppool.tile([P, H, S], bf16, tag="qhT")   # [Dh, h, Sq]
            khT = ppool.tile([P, H, S], bf16, tag="khT")   # [Dh, h, Sk]
            vh = ppool.tile([P, MT, HS], bf16, tag="vh")   # [seq_p, m, 4*Dh]
            oT = ppool.tile([P, H, S], bf16, tag="oT")     # [Dh, h, Sq] normalized
            wo_sb = ppool.tile([P, H, D], bf16, tag="wo_sb")
            dma.dma_start(wo_sb, wo.rearrange("(k p) n -> p k n", p=P))

            # ---------------- projections ----------------
            with (
                tc.sbuf_pool(name="wqkv", bufs=1) as wpool,
                tc.sbuf_pool(name="xs", bufs=20) as xpool,
                tc.psum_pool(name="pproj", bufs=8) as pjp,
            ):
                wq_sb = wpool.tile([P, KD, HS], bf16, tag="wq_sb")
                wk_sb = wpool.tile([P, KD, HS], bf16, tag="wk_sb")
                wv_sb = wpool.tile([P, KD, HS], bf16, tag="wv_sb")
                dma.dma_start(wq_sb, wq.rearrange("(k p) n -> p k n", p=P))
                dma.dma_start(wk_sb, wk.rearrange("(k p) n -> p k n", p=P))
                dma.dma_start(wv_sb, wv.rearrange("(k p) n -> p k n", p=P))

                def proj_qk(x_dram, w_sb, out_sb):
                    # out_sb[:, h, :] = (x^T)^T-contraction: for each head dim tile
                    # lhsT = w_sb[:, kd, h*128:(h+1)*128], rhs = x^T k-slice
                    for nh in range(2):  # S halves, 1024 wide
                        xt = []
                        for kd in range(KD):
                            xti = xpool.tile([P, 1024], bf16, tag="xt")
                            dma.dma_start(
                                xti,
                                x_dram[kd * P:(kd + 1) * P, nh * 1024:(nh + 1) * 1024],
                            )
                            xt.append(xti)
                        pss = [
                            [pjp.tile([P, 512], f32, tag="psproj", name="psproj") for _ in range(2)]
                            for _ in range(H)
                        ]
                        for kd in range(KD):
                            for h in range(H):
                                for n in range(2):
                                    nc.tensor.matmul(
                                        pss[h][n],
                                        lhsT=w_sb[:, kd, h * P:(h + 1) * P],
                                        rhs=xt[kd][:, n * 512:(n + 1) * 512],
                                        start=(kd == 0),
                                        stop=(kd == KD - 1),
                                    )
                        for h in range(H):
                            for n in range(2):
                                dst = out_sb[:, h, nh * 1024 + n * 512: nh * 1024 + (n + 1) * 512]
                                if (h * 2 + n) % 2 == 0:
                                    nc.scalar.copy(dst, pss[h][n])
                                else:
                                    nc.vector.tensor_copy(dst, pss[h][n])

                proj_qk(xq, wq_sb, qhT)
                proj_qk(xk, wk_sb, khT)

                # V projection: Vh [seq, 512]; lhsT = xv^T tile (stationary)
                for nh in range(2):  # seq halves
                    xt2 = []
                    for kd in range(KD):
                        xti = xpool.tile([P, 1024], bf16, tag="xt")
                        dma.dma_start(
                            xti,
                            xv[kd * P:(kd + 1) * P, nh * 1024:(nh + 1) * 1024],
                        )
                        xt2.append(xti)
                    for mg in range(8):
                        m = nh * 8 + mg
                        psv = pjp.tile([P, 512], f32, tag="psproj")
                        for kd in range(KD):
                            nc.tensor.matmul(
                                psv,
                                lhsT=xt2[kd][:, mg * P:(mg + 1) * P],
                                rhs=wv_sb[:, kd, :],
                                start=(kd == 0),
                                stop=(kd == KD - 1),
                            )
                        if m % 2 == 0:
                            nc.scalar.copy(vh[:, m, :], psv)
                        else:
                            nc.vector.tensor_copy(vh[:, m, :], psv)

            # ---------------- attention (per head) ----------------
            with (
                tc.sbuf_pool(name="pts", bufs=24) as ptpool,
                tc.psum_pool(name="pattn", bufs=1) as pap,
            ):
                def normalize(hh, ps_o_hh, ps_d_hh):
                    # oT = ps_o * (1/denom) broadcast across partitions
                    for n in range(N4):
                        d_bf = spool.tile([1, 512], bf16, tag="d_bf")
                        nc.scalar.copy(d_bf, ps_d_hh[32 * n:32 * n + 1, :])
                        ps_b = pap.tile([P, 512], f32, tag="ps_b", bufs=1)
                        nc.tensor.matmul(ps_b, lhsT=ones[0:1, :], rhs=d_bf)
                        rb = spool.tile([P, 512], f32, tag="rb")
                        nc.vector.reciprocal(rb, ps_b)
                        nc.vector.tensor_mul(
                            oT[:, hh, n * 512:(n + 1) * 512], ps_o_hh[n], rb
                        )

                for h in range(H):
                    pt = []  # P^T tiles [Sk_tile, Sq]
                    ps_o = [pap.tile([P, 512], f32, tag=f"ps_o{n}", bufs=1, name=f"ps_o{n}") for n in range(N4)]
                    ps_d = pap.tile([P, 512], f32, tag="ps_d", bufs=1)

                    def scores_half(m, pti, nlo):
                        for n in (nlo, nlo + 1):
                            ps_s = pap.tile([P, 512], f32, tag="ps_s", bufs=2)
                            nc.tensor.matmul(
                                ps_s,
                                lhsT=khT[:, h, m * P:(m + 1) * P],
                                rhs=qhT[:, h, n * 512:(n + 1) * 512],
                            )
                            nc.scalar.activation(
                                pti[:, n * 512:(n + 1) * 512], ps_s, Exp
                            )

                    # software-pipelined: scores/exp for tile m, O/denom for m-2
                    # (interleaved so PE never waits on ACT draining score PSUM)
                    for mstep in range(MT + 2):
                        if mstep < MT:
                            pti = ptpool.tile([P, S], bf16, tag="pt")
                            scores_half(mstep, pti, 0)
                            pt.append(pti)
                        if mstep >= 2:
                            m = mstep - 2
                            for n in range(N4):
                                nc.tensor.matmul(
                                    ps_o[n],
                                    lhsT=vh[:, m, h * P:(h + 1) * P],
                                    rhs=pt[m][:, n * 512:(n + 1) * 512],
                                    start=(m == 0),
                                    stop=(m == MT - 1),
                                )
                        if mstep < MT:
                            scores_half(mstep, pt[mstep], 2)
                        if mstep >= 3 and (mstep - 3) % 2 == 0:
                            # pair-sum exp tiles on DVE (bf16) so the ones-matmul
                            # denominator reduction contracts 8 tiles, not 16
                            j = (mstep - 3) // 2
                            nc.vector.tensor_add(pt[2 * j], pt[2 * j], pt[2 * j + 1])
                            for n in range(N4):
                                nc.tensor.matmul(
                                    ps_d[32 * n:32 * n + 1, :],
                                    lhsT=ones[:, 0:1],
                                    rhs=pt[2 * j][:, n * 512:(n + 1) * 512],
                                    start=(j == 0),
                                    stop=(j == MT // 2 - 1),
                                    tile_position=(0, 32 * n),
                                )
                    normalize(h, ps_o, ps_d)

            # ---------------- output projection ----------------
            with tc.psum_pool(name="pout", bufs=8) as pop:
                for m in range(MT):
                    psf = [pop.tile([P, 512], f32, tag="psout", name="psout") for _ in range(N4)]
                    for kh in range(H):
                        for n in range(N4):
                            nc.tensor.matmul(
                                psf[n],
                                lhsT=oT[:, kh, m * P:(m + 1) * P],
                                rhs=wo_sb[:, kh, n * 512:(n + 1) * 512],
                                start=(kh == 0),
                                stop=(kh == H - 1),
                            )
                    for n in range(N4):
                        ob = opool.tile([P, 512], f32, tag="ob")
                        if n % 2 == 0:
                            nc.scalar.copy(ob, psf[n])
                        else:
                            nc.vector.tensor_copy(ob, psf[n])
                        dma.dma_start(
                            out[m * P:(m + 1) * P, n * 512:(n + 1) * 512], ob
                        )

    nc.compile()
    return nc


def _get_nc():
    if "nc" not in _CACHE:
        _CACHE["nc"] = _build_bass()
    return _CACHE["nc"]


def _prep_inputs(q, k, v, Wq, Wk, Wv, Wo):
    """Host-side sharding: per-core transposed bf16 slices."""
    scale = float(DH) ** -0.5
    q = np.asarray(q, np.float32)
    k = np.asarray(k, np.float32)
    v = np.asarray(v, np.float32)
    Wq = np.asarray(Wq, np.float32)
    Wk = np.asarray(Wk, np.float32)
    Wv = np.asarray(Wv, np.float32)
    Wo = np.asarray(Wo, np.float32)
    in_maps = []
    xT = {}
    for b in range(B):
        xT[b] = (
            q[b].T.astype(BF16),
            k[b].T.astype(BF16),
            v[b].T.astype(BF16),
        )
    for c in range(8):
        b, hg = divmod(c, 4)
        hs = hg * HS
        xqT, xkT, xvT = xT[b]
        in_maps.append(
            {
                "xq": xqT,
                "xk": xkT,
                "xv": xvT,
                "wq": np.ascontiguousarray((Wq[hs:hs + HS, :] * scale).T).astype(BF16),
                "wk": np.ascontiguousarray(Wk[hs:hs + HS, :].T).astype(BF16),
                "wv": np.ascontiguousarray(Wv[hs:hs + HS, :].T).astype(BF16),
                "wo": np.ascontiguousarray(Wo[:, hs:hs + HS].T).astype(BF16),
            }
        )
    return in_maps


def run_spmd(q, k, v, Wq, Wk, Wv, Wo, trace=False):
    from concourse.bass_utils import run_bass_kernel_spmd

    nc = _get_nc()
    in_maps = _prep_inputs(q, k, v, Wq, Wk, Wv, Wo)
    res = run_bass_kernel_spmd(nc, in_maps, list(range(8)), trace=trace)
    out = np.zeros((B, S, D), np.float32)
    for c in range(8):
        out[c // 4] += np.asarray(res.results[c]["out"], np.float32)
    return out, res


def kernel(q, k, v, mask, Wq, Wk, Wv, Wo):
    out, _ = run_spmd(q, k, v, Wq, Wk, Wv, Wo, trace=False)
    return out


# revision 17
# speedup vs baseline: 1.0292x; 1.0142x over previous
"""MHA kernel for 8 Trainium2 NeuronCores.

Reference computation (per batch b):
    Qh = (q[b] @ Wq.T) * Dh^-0.5, Kh = k[b] @ Wk.T, Vh = v[b] @ Wv.T   (split into 16 heads of 128)
    P  = softmax(Qh Kh^T), O = P Vh, out[b] = concat_heads(O) @ Wo.T
Mask is all-False (spec fill=zeros) and is ignored.

Sharding: 8 cores = 2 batches x 4 head-groups (4 heads / core).
Wq/Wk/Wv are split column-wise (output dims), Wo row-wise (input dims);
the all-reduce after the output projection is done on the host during the
gather (sum of the 4 per-head-group partial projections per batch).

Per-core device kernel (all matmul operands bf16, PSUM accumulation fp32):
  inputs (host-prepared): xq/xk/xv = x[b].T [D,S]; wq/wk/wv = W_slice.T [D,512]
  (Dh^-0.5 folded into wq); wo = Wo_slice.T [512, D].
  1) QhT/KhT [Dh,S] per head (head-dim on partitions), Vh [S, 512] (seq on partitions)
  2) per head: scores^T [Sk,Sq] = KhT_m^T.T @ QhT ; P^T = exp(scores^T)
     O^T [Dh,Sq] = sum_m Vh_m.T @ P^T_m ; denom = sum_m ones.T @ P^T_m (M=1 matmuls)
     denom broadcast across partitions via k=1 matmul, reciprocal, normalize O^T
  3) partial out = concat(O^T).T @ wo, accumulated over the 4 heads in PSUM
"""

import numpy as np
import ml_dtypes

BF16 = ml_dtypes.bfloat16

B = 2
S = 2048
D = 2048
NH_TOT = 16
DH = 128
H = 4            # heads per core
HS = H * DH      # 512, model-dim slice per core
P = 128
KD = D // P      # 16 contraction tiles over model dim
MT = S // P      # 16 seq tiles
N4 = S // 512    # 4 column groups of 512

_CACHE: dict = {}


def _build_bass():
    import concourse.tile as tile
    from concourse import bacc, mybir

    f32 = mybir.dt.float32
    bf16 = mybir.dt.bfloat16
    Exp = mybir.ActivationFunctionType.Exp

    nc = bacc.Bacc()

    xq = nc.declare_dram_parameter("xq", [D, S], bf16, isOutput=False)
    xk = nc.declare_dram_parameter("xk", [D, S], bf16, isOutput=False)
    xv = nc.declare_dram_parameter("xv", [D, S], bf16, isOutput=False)
    wq = nc.declare_dram_parameter("wq", [D, HS], bf16, isOutput=False)
    wk = nc.declare_dram_parameter("wk", [D, HS], bf16, isOutput=False)
    wv = nc.declare_dram_parameter("wv", [D, HS], bf16, isOutput=False)
    wo = nc.declare_dram_parameter("wo", [HS, D], bf16, isOutput=False)
    out = nc.declare_dram_parameter("out", [S, D], f32, isOutput=True)

    dma = nc.default_dma_engine

    with tile.TileContext(nc) as tc:
        with (
            tc.sbuf_pool(name="const", bufs=1) as cpool,
            tc.sbuf_pool(name="persist", bufs=1) as ppool,
            tc.sbuf_pool(name="small", bufs=4) as spool,
            tc.sbuf_pool(name="ostage", bufs=8) as opool,
        ):
            ones = cpool.tile([P, P], bf16, tag="ones")
            nc.vector.memset(ones, 1.0)

            qhT = ppool.tile([P, H, S], bf16, tag="qhT")   # [Dh, h, Sq]
            khT = ppool.tile([P, H, S], bf16, tag="khT")   # [Dh, h, Sk]
            vh = ppool.tile([P, MT, HS], bf16, tag="vh")   # [seq_p, m, 4*Dh]
            oT = ppool.tile([P, H, S], bf16, tag="oT")     # [Dh, h, Sq] normalized
            wo_sb = ppool.tile([P, H, D], bf16, tag="wo_sb")
            dma.dma_start(wo_sb, wo.rearrange("(k p) n -> p k n", p=P))

            # ---------------- projections ----------------
            with (
                tc.sbuf_pool(name="wqkv", bufs=1) as wpool,
                tc.sbuf_pool(name="xs", bufs=20) as xpool,
                tc.psum_pool(name="pproj", bufs=8) as pjp,
            ):
                wq_sb = wpool.tile([P, KD, HS], bf16, tag="wq_sb")
                wk_sb = wpool.tile([P, KD, HS], bf16, tag="wk_sb")
                wv_sb = wpool.tile([P, KD, HS], bf16, tag="wv_sb")
                dma.dma_start(wq_sb, wq.rearrange("(k p) n -> p k n", p=P))
                dma.dma_start(wk_sb, wk.rearrange("(k p) n -> p k n", p=P))
                dma.dma_start(wv_sb, wv.rearrange("(k p) n -> p k n", p=P))

                def proj_qk(x_dram, w_sb, out_sb):
                    # out_sb[:, h, :] = (x^T)^T-contraction: for each head dim tile
                    # lhsT = w_sb[:, kd, h*128:(h+1)*128], rhs = x^T k-slice
                    for nh in range(2):  # S halves, 1024 wide
                        xt = []
                        for kd in range(KD):
                            xti = xpool.tile([P, 1024], bf16, tag="xt")
                            dma.dma_start(
                                xti,
                                x_dram[kd * P:(kd + 1) * P, nh * 1024:(nh + 1) * 1024],
                            )
                            xt.append(xti)
                        pss = [
                            [pjp.tile([P, 512], f32, tag="psproj", name="psproj") for _ in range(2)]
                            for _ in range(H)
                        ]
                        for kd in range(KD):
                            for h in range(H):
                                for n in range(2):
                                    nc.tensor.matmul(
                                        pss[h][n],
                                        lhsT=w_sb[:, kd, h * P:(h + 1) * P],
                                        rhs=xt[kd][:, n * 512:(n + 1) * 512],
                                        start=(kd == 0),
                                        stop=(kd == KD - 1),
                                    )
                        for h in range(H):
                            for n in range(2):
                                dst = out_sb[:, h, nh * 1024 + n * 512: nh * 1024 + (n + 1) * 512]
                                if (h * 2 + n) % 2 == 0:
                                    nc.scalar.copy(dst, pss[h][n])
                                else:
                                    nc.vector.tensor_copy(dst, pss[h][n])

                proj_qk(xq, wq_sb, qhT)
                proj_qk(xk, wk_sb, khT)

                # V projection: Vh [seq, 512]; lhsT = xv^T tile (stationary)
                for nh in range(2):  # seq halves
                    xt2 = []
                    for kd in range(KD):
                        xti = xpool.tile([P, 1024], bf16, tag="xt")
                        dma.dma_start(
                            xti,
                            xv[kd * P:(kd + 1) * P, nh * 1024:(nh + 1) * 1024],
                        )
                        xt2.append(xti)
                    for mg in range(8):
                        m = nh * 8 + mg
                        psv = pjp.tile([P, 512], f32, tag="psproj")
                        for kd in range(KD):
                            nc.tensor.matmul(
                                psv,
                                lhsT=xt2[kd][:, mg * P:(mg + 1) * P],
                                rhs=wv_sb[:, kd, :],
                                start=(kd == 0),
                                stop=(kd == KD - 1),
                            )
                        if m % 2 == 0:
                            nc.scalar.copy(vh[:, m, :], psv)
                        else:
                            nc.vector.tensor_copy(vh[:, m, :], psv)

            # ---------------- attention (per head) ----------------
            with (
                tc.sbuf_pool(name="pts", bufs=24) as ptpool,
                tc.psum_pool(name="pattn", bufs=1) as pap,
            ):
                def normalize(hh, ps_o_hh, ps_d_hh):
                    # oT = ps_o * (1/denom) broadcast across partitions
                    for n in range(N4):
                        d_bf = spool.tile([1, 512], bf16, tag="d_bf")
                        nc.scalar.copy(d_bf, ps_d_hh[32 * n:32 * n + 1, :])
                        ps_b = pap.tile([P, 512], f32, tag="ps_b", bufs=1)
                        nc.tensor.matmul(ps_b, lhsT=ones[0:1, :], rhs=d_bf)
                        rb = spool.tile([P, 512], f32, tag="rb")
                        nc.vector.reciprocal(rb, ps_b)
                        nc.vector.tensor_mul(
                            oT[:, hh, n * 512:(n + 1) * 512], ps_o_hh[n], rb
                        )

                for h in range(H):
                    pt = []  # P^T tiles [Sk_tile, Sq]
                    ps_o = [pap.tile([P, 512], f32, tag=f"ps_o{n}", bufs=1, name=f"ps_o{n}") for n in range(N4)]
                    ps_d = pap.tile([P, 512], f32, tag="ps_d", bufs=1)

                    def scores_half(m, pti, nlo):
                        for n in (nlo, nlo + 1):
                            ps_s = pap.tile([P, 512], f32, tag="ps_s", bufs=2)
                            nc.tensor.matmul(
                                ps_s,
                                lhsT=khT[:, h, m * P:(m + 1) * P],
                                rhs=qhT[:, h, n * 512:(n + 1) * 512],
                            )
                            nc.scalar.activation(
                                pti[:, n * 512:(n + 1) * 512], ps_s, Exp
                            )

                    # software-pipelined: scores/exp for tile m, O/denom for m-2
                    # (interleaved so PE never waits on ACT draining score PSUM)
                    for mstep in range(MT + 2):
                        if mstep < MT:
                            pti = ptpool.tile([P, S], bf16, tag="pt")
                            scores_half(mstep, pti, 0)
                            pt.append(pti)
                        if mstep >= 2:
                            m = mstep - 2
                            for n in range(N4):
                                nc.tensor.matmul(
                                    ps_o[n],
                                    lhsT=vh[:, m, h * P:(h + 1) * P],
                                    rhs=pt[m][:, n * 512:(n + 1) * 512],
                                    start=(m == 0),
                                    stop=(m == MT - 1),
                                )
                        if mstep < MT:
                            scores_half(mstep, pt[mstep], 2)
                        if mstep >= 3 and (mstep - 3) % 2 == 0:
                            # pair-sum exp tiles on DVE (bf16) so the ones-matmul
                            # denominator reduction contracts 8 tiles, not 16
                            j = (mstep - 3) // 2
                            nc.vector.tensor_add(pt[2 * j], pt[2 * j], pt[2 * j + 1])
                            for n in range(N4):
                                nc.tensor.matmul(
                                    ps_d[32 * n:32 * n + 1, :],
                                    lhsT=ones[:, 0:1],
                                    rhs=pt[2 * j][:, n * 512:(n + 1) * 512],
                                    start=(j == 0),
                                    stop=(j == MT // 2 - 1),
                                    tile_position=(0, 32 * n),
                                )
                    normalize(h, ps_o, ps_d)

            # ---------------- output projection ----------------
            with tc.psum_pool(name="pout", bufs=8) as pop:
                for m in range(MT):
                    psf = [pop.tile([P, 512], f32, tag="psout", name="psout") for _ in range(N4)]
                    for kh in range(H):
                        for n in range(N4):
                            nc.tensor.matmul(
                                psf[n],
                                lhsT=oT[:, kh, m * P:(m + 1) * P],
                                rhs=wo_sb[:, kh, n * 512:(n + 1) * 512],
                                start=(kh == 0),
                                stop=(kh == H - 1),
                            )
                    for n in range(N4):
                        ob = opool.tile([P, 512], f32, tag="ob")
                        if n % 2 == 0:
                            nc.scalar.copy(ob, psf[n])
                        else:
                            nc.vector.tensor_copy(ob, psf[n])
                        dma.dma_start(
                            out[m * P:(m + 1) * P, n * 512:(n + 1) * 512], ob
                        )

    nc.compile()
    return nc


def _get_nc():
    if "nc" not in _CACHE:
        _CACHE["nc"] = _build_bass()
    return _CACHE["nc"]


def _prep_inputs(q, k, v, Wq, Wk, Wv, Wo):
    """Host-side sharding: per-core transposed bf16 slices."""
    scale = float(DH) ** -0.5
    q = np.asarray(q, np.float32)
    k = np.asarray(k, np.float32)
    v = np.asarray(v, np.float32)
    Wq = np.asarray(Wq, np.float32)
    Wk = np.asarray(Wk, np.float32)
    Wv = np.asarray(Wv, np.float32)
    Wo = np.asarray(Wo, np.float32)
    in_maps = []
    xT = {}
    for b in range(B):
        xT[b] = (
            q[b].T.astype(BF16),
            k[b].T.astype(BF16),
            v[b].T.astype(BF16),
        )
    for c in range(8):
        b, hg = divmod(c, 4)
        hs = hg * HS
        xqT, xkT, xvT = xT[b]
        in_maps.append(
            {
                "xq": xqT,
                "xk": xkT,
                "xv": xvT,
                "wq": np.ascontiguousarray((Wq[hs:hs + HS, :] * scale).T).astype(BF16),
                "wk": np.ascontiguousarray(Wk[hs:hs + HS, :].T).astype(BF16),
                "wv": np.ascontiguousarray(Wv[hs:hs + HS, :].T).astype(BF16),
                "wo": np.ascontiguousarray(Wo[:, hs:hs + HS].T).astype(BF16),
            }
        )
    return in_maps


def run_spmd(q, k, v, Wq, Wk, Wv, Wo, trace=False):
    from concourse.bass_utils import run_bass_kernel_spmd

    nc = _get_nc()
    in_maps = _prep_inputs(q, k, v, Wq, Wk, Wv, Wo)
    res = run_bass_kernel_spmd(nc, in_maps, list(range(8)), trace=trace)
    out = np.zeros((B, S, D), np.float32)
    for c in range(8):
        out[c // 4] += np.asarray(res.results[c]["out"], np.float32)
    return out, res


def kernel(q, k, v, mask, Wq, Wk, Wv, Wo):
    out, _ = run_spmd(q, k, v, Wq, Wk, Wv, Wo, trace=False)
    return out
